# revision 1
# baseline (speedup 1.0000x reference)
"""Trainium2 Bass kernel for windowed cross-attention (nn_CrossAttention_37056977830404).

Sharding: data-parallel over batch B=8 across the 8 NeuronCores (one batch
element per core). All weights replicated. Host-side prep is layout-only
(transposes / dtype casts / folding the avg-pool divisor into Wsr).

Per-core pipeline (all shapes hardcoded):
  y [12544, 256] -> (host yT [256,12544] bf16) -> 2x2 sum-pool -> ypT [256,3136]
  z = yp @ (Wsr/4).T + bsr  (bf16 matmul, fp32 psum)     [sr conv]
  LN over channels (cross-partition ones-matmul sums) + gelu -> y2T bf16
  kT = (y2 @ Wkv_k.T).T     [channel-major, bf16]
  v_w = y2 @ Wkv_v.T        [window-major via windowed stationary APs, bf16]
  qT = (x @ Wq.T).T         [channel-major, bf16]
  per (head, window-row): S^T = k_w^T q_w ; E = exp(S^T/8) ; sums via
  ones-matmul broadcast ; AV = v_w^T E ; attT = AV * recip(sum)  [f32r]
  out = attT.T @ Wproj.T + bproj   (f32r matmuls)
"""
import sys

sys.path.insert(0, '/opt/trn_rl_repo')
import numpy as np

B = 8
C1 = 512
N1 = 3136
NH = 8
HD = 64
WS = 7
C2 = 256
H2 = W2 = 112
HP = WP = 56
NCH = 392      # dense matmul n-chunk (free dim) = one window-row
NCHUNKS = 8    # 3136 / 392
EPS = 1e-5

_cache = {}


def _build_nc():
    import concourse.bacc as bacc
    import concourse.tile as tile
    from concourse import mybir

    F32 = mybir.dt.float32
    F32R = mybir.dt.float32r
    BF16 = mybir.dt.bfloat16
    AF = mybir.ActivationFunctionType

    nc = bacc.Bacc()

    # ---------------- DRAM I/O ----------------
    xT = nc.dram_tensor("xT", [C1, N1], BF16, kind="ExternalInput")
    yT = nc.dram_tensor("yT", [C2, H2 * W2], BF16, kind="ExternalInput")
    WqT = nc.dram_tensor("WqT", [C1, C1], BF16, kind="ExternalInput")
    WsrT = nc.dram_tensor("WsrT", [C2, C2], BF16, kind="ExternalInput")  # pre-scaled 1/4
    WkvT = nc.dram_tensor("WkvT", [C2, 2 * C1], BF16, kind="ExternalInput")
    WpT = nc.dram_tensor("WpT", [C1, C1], F32R, kind="ExternalInput")
    bsr = nc.dram_tensor("bsr", [C2], F32, kind="ExternalInput")
    gnr = nc.dram_tensor("gnr", [2, 128], F32R, kind="ExternalInput")  # gn as rows
    bnc = nc.dram_tensor("bnc", [C2], F32, kind="ExternalInput")
    bp = nc.dram_tensor("bp", [1, C1], F32R, kind="ExternalInput")
    out = nc.dram_tensor("out", [N1, C1], F32, kind="ExternalOutput")

    with tile.TileContext(nc) as tc:
        _emit(nc, tc, mybir, F32, F32R, BF16, AF,
              xT, yT, WqT, WsrT, WkvT, WpT, bsr, gnr, bnc, bp, out)
    nc.finalize()
    return nc


def _emit(nc, tc, mybir, F32, F32R, BF16, AF,
          xT, yT, WqT, WsrT, WkvT, WpT, bsr, gnr, bnc, bp, out):
    from contextlib import ExitStack

    with ExitStack() as ctx:
        pool_w = ctx.enter_context(tc.tile_pool(name="pool_w", bufs=1))
        pool_big = ctx.enter_context(tc.tile_pool(name="pool_big", bufs=1))
        pool_vw = ctx.enter_context(tc.tile_pool(name="pool_vw", bufs=2))
        pool_tmp = ctx.enter_context(tc.tile_pool(name="pool_tmp", bufs=2))

        # ---------------- weights / constants to SBUF ----------------
        wq, wp, wsr, wkv = [], [], [], []
        for ct in range(4):
            wq_t = pool_w.tile([128, C1], BF16, name=f"wq{ct}", tag=f"wq{ct}")
            nc.sync.dma_start(out=wq_t, in_=WqT[ct * 128:(ct + 1) * 128, :])
            wq.append(wq_t)
            wp_t = pool_w.tile([128, C1], F32R, name=f"wp{ct}", tag=f"wp{ct}")
            nc.sync.dma_start(out=wp_t, in_=WpT[ct * 128:(ct + 1) * 128, :])
            wp.append(wp_t)
        for kt in range(2):
            wsr_t = pool_w.tile([128, C2], BF16, name=f"wsr{kt}", tag=f"wsr{kt}")
            nc.sync.dma_start(out=wsr_t, in_=WsrT[kt * 128:(kt + 1) * 128, :])
            wsr.append(wsr_t)
            wkv_t = pool_w.tile([128, 2 * C1], BF16, name=f"wkv{kt}", tag=f"wkv{kt}")
            nc.sync.dma_start(out=wkv_t, in_=WkvT[kt * 128:(kt + 1) * 128, :])
            wkv.append(wkv_t)
        bsr_c, bn_c, gn_r = [], [], []
        for ot in range(2):
            b1 = pool_w.tile([128, 1], F32, name=f"bsr{ot}", tag=f"bsr{ot}")
            nc.sync.dma_start(out=b1, in_=bsr[ot * 128:(ot + 1) * 128].unsqueeze(1))
            bsr_c.append(b1)
            b2 = pool_w.tile([128, 1], F32, name=f"bn{ot}", tag=f"bn{ot}")
            nc.sync.dma_start(out=b2, in_=bnc[ot * 128:(ot + 1) * 128].unsqueeze(1))
            bn_c.append(b2)
            g1 = pool_w.tile([1, 128], F32R, name=f"gnr{ot}", tag=f"gnr{ot}")
            nc.sync.dma_start(out=g1, in_=gnr[ot:ot + 1, :])
            gn_r.append(g1)
        bp_sb = pool_w.tile([1, C1], F32R, name="bp_sb", tag="bp_sb")
        nc.sync.dma_start(out=bp_sb, in_=bp.ap())

        ones_f = pool_w.tile([128, 1], F32, name="ones_f", tag="ones_f")
        nc.vector.memset(ones_f, 1.0)
        ones_c = pool_w.tile([128, 1], F32R, name="ones_c", tag="ones_c")
        nc.vector.tensor_copy(ones_c[:], ones_f[:])
        ones_rf = pool_w.tile([1, 128], F32, name="ones_rf", tag="ones_rf")
        nc.vector.memset(ones_rf, 1.0)
        ones_r = pool_w.tile([1, 128], F32R, name="ones_r", tag="ones_r")
        nc.vector.tensor_copy(ones_r[:], ones_rf[:])
        ones_s = pool_w.tile([49, 64], BF16, name="ones_s", tag="ones_s")
        nc.vector.memset(ones_s, 1.0)
        eps_sb = pool_w.tile([1, 1], F32, name="eps_sb", tag="eps_sb")
        nc.vector.memset(eps_sb, EPS)

        # ---------------- persistent activations ----------------
        y2T = [pool_big.tile([128, N1], BF16, name=f"y2T{k}", tag=f"y2T{k}")
               for k in range(2)]
        kT = [pool_big.tile([128, N1], BF16, name=f"kT{t}", tag=f"kT{t}")
              for t in range(4)]
        qT = [pool_big.tile([128, N1], BF16, name=f"qT{t}", tag=f"qT{t}")
              for t in range(4)]

        with tc.tile_pool(name="pool_yp", bufs=1) as pool_yp, \
             tc.tile_pool(name="ps_d", bufs=2, space="PSUM") as ps_d:
            ypT = [pool_yp.tile([128, N1], BF16, name=f"ypT{k}", tag=f"ypT{k}")
                   for k in range(2)]

            # ------------ stage 1: 2x2 sum-pool of y ------------
            # yT [256, 12544]; spatial 112x112. chunk = 14 in rows -> 7 out rows
            # (= one window-row wi). Output is written WINDOW-MAJOR:
            # col n' = (wi*8 + wj)*49 + i*7 + j.
            for kt in range(2):
                for wi in range(8):
                    yin = pool_tmp.tile([128, 14 * 112], BF16, name="yin",
                                        tag="yin", bufs=3)
                    nc.sync.dma_start(
                        out=yin,
                        in_=yT[kt * 128:(kt + 1) * 128,
                               wi * 14 * 112:(wi + 1) * 14 * 112])
                    yv = yin.rearrange("p (r j two) -> p r j two",
                                       r=14, j=56, two=2)
                    hp = pool_tmp.tile([128, 14 * 56], BF16, name="hp",
                                       tag="hp", bufs=2)
                    hv = hp.rearrange("p (r j) -> p r j", r=14, j=56)
                    nc.gpsimd.tensor_add(hv, yv[:, :, :, 0], yv[:, :, :, 1])
                    # vertical add of row pairs + window-major scatter write
                    hv2 = hp.rearrange("p (i two b j) -> p i two b j",
                                       i=7, two=2, b=8, j=7)
                    ov = ypT[kt][:, wi * 392:(wi + 1) * 392].rearrange(
                        "p (b i j) -> p i b j", b=8, i=7, j=7)
                    nc.vector.tensor_add(ov, hv2[:, :, 0, :, :],
                                         hv2[:, :, 1, :, :])

            # ------------ stage 2: sr conv + LN + gelu ------------
            for ch in range(NCHUNKS):
                cs = slice(ch * NCH, (ch + 1) * NCH)
                zsb = []
                for ot in range(2):
                    pz = ps_d.tile([128, NCH], F32, name="pz", tag="pz")
                    for kt in range(2):
                        nc.tensor.matmul(pz[:], wsr[kt][:, ot * 128:(ot + 1) * 128],
                                         ypT[kt][:, cs],
                                         start=(kt == 0), stop=(kt == 1))
                    z_t = pool_tmp.tile([128, NCH], F32R, name="z_t",
                                        tag="zsb", bufs=4)
                    nc.scalar.activation(out=z_t[:], in_=pz[:], func=AF.Identity,
                                         bias=bsr_c[ot])
                    zsb.append(z_t)
                pst_s = ps_d.tile([1, NCH], F32, name="pst_s", tag="pst_s", bufs=1)
                pst_q = ps_d.tile([1, NCH], F32, name="pst_q", tag="pst_q", bufs=1)
                for ot in range(2):
                    nc.tensor.matmul(pst_s[:], ones_c[:], zsb[ot][:],
                                     start=(ot == 0), stop=(ot == 1))
                for ot in range(2):
                    zq = pool_tmp.tile([128, NCH], F32R, name="zq", tag="zq", bufs=2)
                    nc.scalar.activation(out=zq[:], in_=zsb[ot][:], func=AF.Square)
                    nc.tensor.matmul(pst_q[:], ones_c[:], zq[:],
                                     start=(ot == 0), stop=(ot == 1))
                m_sb = pool_tmp.tile([1, NCH], F32, name="m_sb", tag="m_sb", bufs=1)
                nc.vector.tensor_scalar_mul(m_sb[:], pst_s[:], 1.0 / C2)
                q_sb = pool_tmp.tile([1, NCH], F32, name="q_sb", tag="q_sb", bufs=1)
                nc.vector.tensor_scalar_mul(q_sb[:], pst_q[:], 1.0 / C2)
                var_sb = pool_tmp.tile([1, NCH], F32, name="var_sb",
                                       tag="var_sb", bufs=1)
                nc.gpsimd.tensor_tensor(var_sb[:], m_sb[:], m_sb[:],
                                        op=mybir.AluOpType.mult)
                nc.gpsimd.tensor_tensor(var_sb[:], q_sb[:], var_sb[:],
                                        op=mybir.AluOpType.subtract)
                sd_sb = pool_tmp.tile([1, NCH], F32, name="sd_sb",
                                      tag="sd_sb", bufs=1)
                nc.scalar.activation(out=sd_sb[:], in_=var_sb[:], func=AF.Sqrt,
                                     bias=eps_sb[:])
                r_sb = pool_tmp.tile([1, NCH], F32R, name="r_sb", tag="r_sb", bufs=1)
                with nc.allow_low_precision(reason="f32r rstd feeds f32r matmul"):
                    nc.vector.reciprocal(out=r_sb[:], in_=sd_sb[:])
                nb_sb = pool_tmp.tile([1, NCH], F32R, name="nb_sb",
                                      tag="nb_sb", bufs=1)
                nc.gpsimd.tensor_tensor(nb_sb[:], m_sb[:], r_sb[:],
                                        op=mybir.AluOpType.mult)
                nc.gpsimd.tensor_scalar_mul(nb_sb[:], nb_sb[:], -1.0)
                for ot in range(2):
                    pa = ps_d.tile([128, NCH], F32, name="pa", tag="pa")
                    nc.tensor.matmul(pa[:], gn_r[ot][:], r_sb[:],
                                     start=True, stop=True)
                    pb = ps_d.tile([128, NCH], F32, name="pb", tag="pb")
                    nc.tensor.matmul(pb[:], gn_r[ot][:], nb_sb[:],
                                     start=True, stop=True)
                    t1 = pool_tmp.tile([128, NCH], F32, name="t1", tag="t1", bufs=2)
                    nc.vector.tensor_mul(t1[:], zsb[ot][:], pa[:])
                    nc.vector.tensor_add(t1[:], t1[:], pb[:])
                    nc.scalar.activation(out=y2T[ot][:, cs], in_=t1[:],
                                         func=AF.Gelu, bias=bn_c[ot])

            # ------------ stage 3: k projection (channel-major) ------------
            for ch in range(NCHUNKS):
                cs = slice(ch * NCH, (ch + 1) * NCH)
                for ot in range(4):
                    pk = ps_d.tile([128, NCH], F32, name="pk", tag="pz")
                    for kt in range(2):
                        nc.tensor.matmul(pk[:],
                                         wkv[kt][:, ot * 128:(ot + 1) * 128],
                                         y2T[kt][:, cs],
                                         start=(kt == 0), stop=(kt == 1))
                    nc.any.tensor_copy(kT[ot][:, cs], pk[:])

            # ------------ stage 4: q projection (channel-major) ------------
            for ch in range(NCHUNKS):
                cs = slice(ch * NCH, (ch + 1) * NCH)
                xin = []
                for ct in range(4):
                    x_t = pool_tmp.tile([128, NCH], BF16, name="x_t",
                                        tag="xin", bufs=6)
                    nc.sync.dma_start(out=x_t,
                                      in_=xT[ct * 128:(ct + 1) * 128, cs])
                    xin.append(x_t)
                for ot in range(4):
                    pq = ps_d.tile([128, NCH], F32, name="pq", tag="pz")
                    for ct in range(4):
                        nc.tensor.matmul(pq[:],
                                         wq[ct][:, ot * 128:(ot + 1) * 128],
                                         xin[ct][:],
                                         start=(ct == 0), stop=(ct == 3))
                    nc.any.tensor_copy(qT[ot][:, cs], pq[:])

        # ------------ stage 5-7: v (window-major), attention, proj ------------
        # qT/kT/y2T columns are window-major: window w = wi*8+wj occupies
        # cols w*49:(w+1)*49. attT stays spatial-major (scatter on write).

        def win_view(t):
            return t.rearrange("p (a i b j) -> p a b i j", a=8, i=7, b=8, j=7)

        with tc.tile_pool(name="pool_att", bufs=1) as pool_att, \
             tc.tile_pool(name="ps_a", bufs=2, space="PSUM") as ps_a:
            attT = [pool_att.tile([128, N1], F32R, name=f"attT{t}", tag=f"attT{t}")
                    for t in range(4)]
            for wi in range(8):
                vw = pool_vw.tile([49, 8 * C1], BF16, name="vw", tag="vw")
                for wj in range(8):
                    wsl = slice((wi * 8 + wj) * 49, (wi * 8 + wj + 1) * 49)
                    pv = ps_a.tile([49, C1], F32, name="pv", tag="pv")
                    for kt in range(2):
                        nc.tensor.matmul(pv[:], y2T[kt][:, wsl],
                                         wkv[kt][:, C1:2 * C1],
                                         start=(kt == 0), stop=(kt == 1))
                    nc.scalar.copy(out=vw[:, wj * C1:(wj + 1) * C1], in_=pv[:])
                for h in range(8):
                    t, pb_ = h // 2, (h % 2) * 64
                    psl = slice(pb_, pb_ + 64)
                    S = ps_a.tile([49, 392], F32, name="S", tag="S")
                    for wj in range(8):
                        wsl = slice((wi * 8 + wj) * 49, (wi * 8 + wj + 1) * 49)
                        nc.tensor.matmul(S[:, wj * 49:(wj + 1) * 49],
                                         kT[t][psl, wsl],
                                         qT[t][psl, wsl],
                                         start=True, stop=True)
                    E = pool_tmp.tile([49, 392], BF16, name="E", tag="E", bufs=3)
                    nc.scalar.activation(out=E[:], in_=S[:], func=AF.Exp,
                                         scale=0.125)
                    SUMB = ps_a.tile([64, 392], F32, name="SUMB",
                                     tag="SUMB", bufs=1)
                    nc.tensor.matmul(SUMB[:], ones_s[:], E[:],
                                     start=True, stop=True)
                    RB = pool_tmp.tile([64, 392], F32, name="RB", tag="RB", bufs=3)
                    nc.vector.reciprocal(out=RB[:], in_=SUMB[:])
                    AV = ps_a.tile([64, 392], F32, name="AV", tag="AV")
                    for wj in range(8):
                        nc.tensor.matmul(
                            AV[:, wj * 49:(wj + 1) * 49],
                            vw[:, wj * C1 + h * 64:wj * C1 + (h + 1) * 64],
                            E[:, wj * 49:(wj + 1) * 49],
                            start=True, stop=True)
                    avv = AV.rearrange("p (b i j) -> p b i j", b=8, i=7, j=7)
                    rbv = RB.rearrange("p (b i j) -> p b i j", b=8, i=7, j=7)
                    nc.vector.tensor_mul(win_view(attT[t])[psl, wi],
                                         avv[:], rbv[:])

            # ------------ stage 7: output projection ------------
            for nt in range(25):
                nsz = min(128, N1 - nt * 128)
                ns = slice(nt * 128, nt * 128 + nsz)
                po = ps_a.tile([128, C1], F32, name="po", tag="pv")
                for ct in range(4):
                    nc.tensor.matmul(po[:nsz, :], attT[ct][:, ns], wp[ct][:],
                                     start=(ct == 0), stop=False)
                nc.tensor.matmul(po[:nsz, :], ones_r[:, :nsz], bp_sb[:],
                                 start=False, stop=True)
                o_sb = pool_tmp.tile([128, C1], F32, name="o_sb",
                                     tag="o_sb", bufs=2)
                nc.any.tensor_copy(o_sb[:nsz, :], po[:nsz, :])
                nc.sync.dma_start(out=out[ns, :], in_=o_sb[:nsz, :])


def _get_nc():
    if "nc" not in _cache:
        _cache["nc"] = _build_nc()
    return _cache["nc"]


def kernel(**inputs):
    import ml_dtypes
    bf16 = ml_dtypes.bfloat16
    f32 = np.float32

    x = np.asarray(inputs["x"], dtype=f32)
    y = np.asarray(inputs["y"], dtype=f32)
    Wq = np.asarray(inputs["Wq"], dtype=f32)
    Wkv = np.asarray(inputs["Wkv"], dtype=f32)
    Wproj = np.asarray(inputs["Wproj"], dtype=f32)
    bproj = np.asarray(inputs["bproj"], dtype=f32)
    Wsr = np.asarray(inputs["Wsr"], dtype=f32)
    bsr_np = np.asarray(inputs["bsr"], dtype=f32)
    gn = np.asarray(inputs["gn"], dtype=f32)
    bn = np.asarray(inputs["bn"], dtype=f32)

    def to_window_major(a2d):
        # cols: spatial n = (wi*7+i)*56 + wj*7+j  ->  n' = (wi*8+wj)*49 + i*7+j
        c = a2d.shape[0]
        v = a2d.reshape(c, 8, 7, 8, 7).transpose(0, 1, 3, 2, 4)
        return np.ascontiguousarray(v.reshape(c, N1))

    WqT = np.ascontiguousarray(Wq.T).astype(bf16)
    WsrT = np.ascontiguousarray(0.25 * Wsr.T).astype(bf16)
    WkvT = np.ascontiguousarray(Wkv.T).astype(bf16)
    WpT = np.ascontiguousarray(Wproj.T).astype(f32)
    gnr = np.ascontiguousarray(gn.reshape(2, 128)).astype(f32)
    bp = np.ascontiguousarray(bproj.reshape(1, C1)).astype(f32)

    nc = _get_nc()
    in_maps = []
    for b in range(B):
        in_maps.append({
            "xT": to_window_major(x[b].T).astype(bf16),
            "yT": np.ascontiguousarray(y[b].T).astype(bf16),
            "WqT": WqT, "WsrT": WsrT, "WkvT": WkvT, "WpT": WpT,
            "bsr": bsr_np, "gnr": gnr, "bnc": bn, "bp": bp,
        })
    from concourse.bass_utils import run_bass_kernel_spmd
    res = run_bass_kernel_spmd(nc, in_maps, core_ids=list(range(B)),
                               **_cache.get("run_opts", {}))
    _cache["last_res"] = res
    return np.stack([r["out"] for r in res.results], axis=0).astype(f32)



# revision 5
# speedup vs baseline: 2.8501x; 2.8501x over previous
"""Trainium2 Bass kernel for windowed cross-attention (nn_CrossAttention_37056977830404).

Sharding: data-parallel over batch B=8 across the 8 NeuronCores (one batch
element per core). All weights replicated.

The axon tunnel to the device is the bottleneck (~55 MB/s), so the host does
the cheap byte-reducing prep and the device does everything else:
  host: 2x2 sum-pool of y (51->13 MB), x cast to fp8e4m3 (51->13 MB),
        window-major row permutation of both, weights cast to bf16,
        output fetched as bf16 (51->26 MB).
  device (per core):
    transpose x [3136,512] fp8 -> xT bf16 [512,3136] (PE identity matmuls)
    transpose yp [3136,256] bf16 -> ypT [256,3136]
    z = yp @ (Wsr/4).T + bsr ; LN over channels + gelu -> y2T bf16
    kT = (y2 @ Wkv_k.T).T ; qT = (x @ Wq.T).T   [channel-major matmul outputs]
    per (head, window): S^T = k_w^T q_w ; E = exp(S^T/8) ; ones-matmul sums ;
    AV = v_w^T E ; attT = AV * recip(sum) -> bf16
    out = attT.T @ Wproj.T + bproj  (bf16 matmuls, f32 psum) -> bf16 out
"""
import sys

sys.path.insert(0, '/opt/trn_rl_repo')
import numpy as np

B = 8
C1 = 512
N1 = 3136
NH = 8
HD = 64
WS = 7
C2 = 256
H2 = W2 = 112
HP = WP = 56
NCH = 392      # dense matmul n-chunk (free dim) = one window-row
NCHUNKS = 8    # 3136 / 392
EPS = 1e-5

_cache = {}


def _build_nc():
    import concourse.bacc as bacc
    import concourse.tile as tile
    from concourse import mybir

    F32 = mybir.dt.float32
    F32R = mybir.dt.float32r
    BF16 = mybir.dt.bfloat16
    F8 = mybir.dt.float8e4
    AF = mybir.ActivationFunctionType

    nc = bacc.Bacc()

    # ---------------- DRAM I/O ----------------
    x8 = nc.dram_tensor("x8", [N1, C1], F8, kind="ExternalInput")
    yp = nc.dram_tensor("yp", [N1, C2], BF16, kind="ExternalInput")
    WqT = nc.dram_tensor("WqT", [C1, C1], BF16, kind="ExternalInput")
    WsrT = nc.dram_tensor("WsrT", [C2, C2], BF16, kind="ExternalInput")  # pre-scaled 1/4
    WkvT = nc.dram_tensor("WkvT", [C2, 2 * C1], BF16, kind="ExternalInput")
    WpT = nc.dram_tensor("WpT", [C1, C1], BF16, kind="ExternalInput")
    bsr = nc.dram_tensor("bsr", [C2], F32, kind="ExternalInput")
    gnr = nc.dram_tensor("gnr", [2, 128], F32R, kind="ExternalInput")  # gn as rows
    bnc = nc.dram_tensor("bnc", [C2], F32, kind="ExternalInput")
    bp = nc.dram_tensor("bp", [1, C1], BF16, kind="ExternalInput")
    ident = nc.dram_tensor("ident", [128, 128], BF16, kind="ExternalInput")
    out = nc.dram_tensor("out", [N1, C1], BF16, kind="ExternalOutput")

    with tile.TileContext(nc) as tc:
        _emit(nc, tc, mybir, F32, F32R, BF16, F8, AF,
              x8, yp, WqT, WsrT, WkvT, WpT, bsr, gnr, bnc, bp, ident, out)
    nc.finalize()
    return nc


def _emit(nc, tc, mybir, F32, F32R, BF16, F8, AF,
          x8, yp, WqT, WsrT, WkvT, WpT, bsr, gnr, bnc, bp, ident, out):
    from contextlib import ExitStack

    with ExitStack() as ctx:
        pool_w = ctx.enter_context(tc.tile_pool(name="pool_w", bufs=1))
        pool_big = ctx.enter_context(tc.tile_pool(name="pool_big", bufs=1))
        pool_vw = ctx.enter_context(tc.tile_pool(name="pool_vw", bufs=2))
        pool_tmp = ctx.enter_context(tc.tile_pool(name="pool_tmp", bufs=2))

        # ---------------- weights / constants to SBUF ----------------
        wq, wp, wsr, wkv = [], [], [], []
        for ct in range(4):
            wq_t = pool_w.tile([128, C1], BF16, name=f"wq{ct}", tag=f"wq{ct}")
            nc.sync.dma_start(out=wq_t, in_=WqT[ct * 128:(ct + 1) * 128, :])
            wq.append(wq_t)
            wp_t = pool_w.tile([128, C1], BF16, name=f"wp{ct}", tag=f"wp{ct}")
            nc.sync.dma_start(out=wp_t, in_=WpT[ct * 128:(ct + 1) * 128, :])
            wp.append(wp_t)
        for kt in range(2):
            wsr_t = pool_w.tile([128, C2], BF16, name=f"wsr{kt}", tag=f"wsr{kt}")
            nc.sync.dma_start(out=wsr_t, in_=WsrT[kt * 128:(kt + 1) * 128, :])
            wsr.append(wsr_t)
            wkv_t = pool_w.tile([128, 2 * C1], BF16, name=f"wkv{kt}", tag=f"wkv{kt}")
            nc.sync.dma_start(out=wkv_t, in_=WkvT[kt * 128:(kt + 1) * 128, :])
            wkv.append(wkv_t)
        bsr_c, bn_c, gn_r = [], [], []
        for ot in range(2):
            b1 = pool_w.tile([128, 1], F32, name=f"bsr{ot}", tag=f"bsr{ot}")
            nc.sync.dma_start(out=b1, in_=bsr[ot * 128:(ot + 1) * 128].unsqueeze(1))
            bsr_c.append(b1)
            b2 = pool_w.tile([128, 1], F32, name=f"bn{ot}", tag=f"bn{ot}")
            nc.sync.dma_start(out=b2, in_=bnc[ot * 128:(ot + 1) * 128].unsqueeze(1))
            bn_c.append(b2)
            g1 = pool_w.tile([1, 128], F32R, name=f"gnr{ot}", tag=f"gnr{ot}")
            nc.sync.dma_start(out=g1, in_=gnr[ot:ot + 1, :])
            gn_r.append(g1)
        bp_sb = pool_w.tile([1, C1], BF16, name="bp_sb", tag="bp_sb")
        nc.sync.dma_start(out=bp_sb, in_=bp.ap())
        id_sb = pool_w.tile([128, 128], BF16, name="id_sb", tag="id_sb")
        nc.sync.dma_start(out=id_sb, in_=ident.ap())

        ones_f = pool_w.tile([128, 1], F32, name="ones_f", tag="ones_f")
        nc.vector.memset(ones_f, 1.0)
        ones_c = pool_w.tile([128, 1], F32R, name="ones_c", tag="ones_c")
        nc.vector.tensor_copy(ones_c[:], ones_f[:])
        ones_rf = pool_w.tile([1, 128], F32, name="ones_rf", tag="ones_rf")
        nc.vector.memset(ones_rf, 1.0)
        ones_r = pool_w.tile([1, 128], BF16, name="ones_r", tag="ones_r")
        nc.vector.tensor_copy(ones_r[:], ones_rf[:])
        ones_s = pool_w.tile([49, 64], BF16, name="ones_s", tag="ones_s")
        nc.vector.memset(ones_s, 1.0)
        eps_sb = pool_w.tile([1, 1], F32, name="eps_sb", tag="eps_sb")
        nc.vector.memset(eps_sb, EPS)

        # ---------------- persistent activations ----------------
        y2T = [pool_big.tile([128, N1], BF16, name=f"y2T{k}", tag=f"y2T{k}")
               for k in range(2)]
        kT = [pool_big.tile([128, N1], BF16, name=f"kT{t}", tag=f"kT{t}")
              for t in range(4)]
        qT = [pool_big.tile([128, N1], BF16, name=f"qT{t}", tag=f"qT{t}")
              for t in range(4)]

        with tc.tile_pool(name="pool_in", bufs=1) as pool_in:
            ypT = [pool_in.tile([128, N1], BF16, name=f"ypT{k}", tag=f"ypT{k}")
                   for k in range(2)]
            xT = [pool_in.tile([128, N1], BF16, name=f"xT{t}", tag=f"xT{t}")
                  for t in range(4)]

            # ------------ stage 0: on-device transposes ------------
            # x8 [3136, 512] fp8 row-major (window-major rows) -> xT bf16
            # yp [3136, 256] bf16 -> ypT bf16. 25 chunks of <=128 rows.
            with tc.tile_pool(name="ps_t", bufs=4, space="PSUM") as ps_t:
                for nt in range(25):
                    nsz = min(128, N1 - nt * 128)
                    ns = slice(nt * 128, nt * 128 + nsz)
                    x_in = pool_tmp.tile([128, C1], F8, name="x_in",
                                         tag="x_in", bufs=3)
                    nc.sync.dma_start(out=x_in[:nsz, :], in_=x8[ns, :])
                    x_bf = pool_tmp.tile([128, C1], BF16, name="x_bf",
                                         tag="x_bf", bufs=3)
                    nc.any.tensor_copy(x_bf[:nsz, :], x_in[:nsz, :])
                    y_in = pool_tmp.tile([128, C2], BF16, name="y_in",
                                         tag="y_in", bufs=3)
                    nc.sync.dma_start(out=y_in[:nsz, :], in_=yp[ns, :])
                    for ct in range(4):
                        pt = ps_t.tile([128, 128], BF16, name="pt", tag="pt")
                        nc.tensor.transpose(pt[:, :nsz],
                                            x_bf[:nsz, ct * 128:(ct + 1) * 128],
                                            id_sb[:nsz, :nsz])
                        nc.any.tensor_copy(xT[ct][:, ns], pt[:, :nsz])
                    for kt in range(2):
                        pt2 = ps_t.tile([128, 128], BF16, name="pt2", tag="pt")
                        nc.tensor.transpose(pt2[:, :nsz],
                                            y_in[:nsz, kt * 128:(kt + 1) * 128],
                                            id_sb[:nsz, :nsz])
                        nc.any.tensor_copy(ypT[kt][:, ns], pt2[:, :nsz])

            with tc.tile_pool(name="ps_d", bufs=2, space="PSUM") as ps_d:
                # ------------ stage 1: sr conv + LN + gelu ------------
                for ch in range(NCHUNKS):
                    cs = slice(ch * NCH, (ch + 1) * NCH)
                    zsb = []
                    for ot in range(2):
                        pz = ps_d.tile([128, NCH], F32, name="pz", tag="pz")
                        for kt in range(2):
                            nc.tensor.matmul(pz[:],
                                             wsr[kt][:, ot * 128:(ot + 1) * 128],
                                             ypT[kt][:, cs],
                                             start=(kt == 0), stop=(kt == 1))
                        z_t = pool_tmp.tile([128, NCH], F32R, name="z_t",
                                            tag="zsb", bufs=4)
                        nc.scalar.activation(out=z_t[:], in_=pz[:],
                                             func=AF.Identity, bias=bsr_c[ot])
                        zsb.append(z_t)
                    pst_s = ps_d.tile([1, NCH], F32, name="pst_s",
                                      tag="pst_s", bufs=1)
                    pst_q = ps_d.tile([1, NCH], F32, name="pst_q",
                                      tag="pst_q", bufs=1)
                    for ot in range(2):
                        nc.tensor.matmul(pst_s[:], ones_c[:], zsb[ot][:],
                                         start=(ot == 0), stop=(ot == 1))
                    for ot in range(2):
                        zq = pool_tmp.tile([128, NCH], F32R, name="zq",
                                           tag="zq", bufs=2)
                        nc.scalar.activation(out=zq[:], in_=zsb[ot][:],
                                             func=AF.Square)
                        nc.tensor.matmul(pst_q[:], ones_c[:], zq[:],
                                         start=(ot == 0), stop=(ot == 1))
                    m_sb = pool_tmp.tile([1, NCH], F32, name="m_sb",
                                         tag="m_sb", bufs=1)
                    nc.vector.tensor_scalar_mul(m_sb[:], pst_s[:], 1.0 / C2)
                    q_sb = pool_tmp.tile([1, NCH], F32, name="q_sb",
                                         tag="q_sb", bufs=1)
                    nc.vector.tensor_scalar_mul(q_sb[:], pst_q[:], 1.0 / C2)
                    var_sb = pool_tmp.tile([1, NCH], F32, name="var_sb",
                                           tag="var_sb", bufs=1)
                    nc.gpsimd.tensor_tensor(var_sb[:], m_sb[:], m_sb[:],
                                            op=mybir.AluOpType.mult)
                    nc.gpsimd.tensor_tensor(var_sb[:], q_sb[:], var_sb[:],
                                            op=mybir.AluOpType.subtract)
                    sd_sb = pool_tmp.tile([1, NCH], F32, name="sd_sb",
                                          tag="sd_sb", bufs=1)
                    nc.scalar.activation(out=sd_sb[:], in_=var_sb[:],
                                         func=AF.Sqrt, bias=eps_sb[:])
                    r_sb = pool_tmp.tile([1, NCH], F32R, name="r_sb",
                                         tag="r_sb", bufs=1)
                    with nc.allow_low_precision(reason="f32r rstd, f32r matmul"):
                        nc.vector.reciprocal(out=r_sb[:], in_=sd_sb[:])
                    nb_sb = pool_tmp.tile([1, NCH], F32R, name="nb_sb",
                                          tag="nb_sb", bufs=1)
                    nc.gpsimd.tensor_tensor(nb_sb[:], m_sb[:], r_sb[:],
                                            op=mybir.AluOpType.mult)
                    nc.gpsimd.tensor_scalar_mul(nb_sb[:], nb_sb[:], -1.0)
                    for ot in range(2):
                        pa = ps_d.tile([128, NCH], F32, name="pa", tag="pa")
                        nc.tensor.matmul(pa[:], gn_r[ot][:], r_sb[:],
                                         start=True, stop=True)
                        pb = ps_d.tile([128, NCH], F32, name="pb", tag="pb")
                        nc.tensor.matmul(pb[:], gn_r[ot][:], nb_sb[:],
                                         start=True, stop=True)
                        t1 = pool_tmp.tile([128, NCH], F32, name="t1",
                                           tag="t1", bufs=2)
                        nc.vector.tensor_mul(t1[:], zsb[ot][:], pa[:])
                        nc.vector.tensor_add(t1[:], t1[:], pb[:])
                        nc.scalar.activation(out=y2T[ot][:, cs], in_=t1[:],
                                             func=AF.Gelu, bias=bn_c[ot])

                # ------------ stage 2: k projection (channel-major) ------------
                for ch in range(NCHUNKS):
                    cs = slice(ch * NCH, (ch + 1) * NCH)
                    for ot in range(4):
                        pk = ps_d.tile([128, NCH], F32, name="pk", tag="pz")
                        for kt in range(2):
                            nc.tensor.matmul(pk[:],
                                             wkv[kt][:, ot * 128:(ot + 1) * 128],
                                             y2T[kt][:, cs],
                                             start=(kt == 0), stop=(kt == 1))
                        nc.any.tensor_copy(kT[ot][:, cs], pk[:])

                # ------------ stage 3: q projection (channel-major) ------------
                for ch in range(NCHUNKS):
                    cs = slice(ch * NCH, (ch + 1) * NCH)
                    for ot in range(4):
                        pq = ps_d.tile([128, NCH], F32, name="pq", tag="pz")
                        for ct in range(4):
                            nc.tensor.matmul(pq[:],
                                             wq[ct][:, ot * 128:(ot + 1) * 128],
                                             xT[ct][:, cs],
                                             start=(ct == 0), stop=(ct == 3))
                        nc.any.tensor_copy(qT[ot][:, cs], pq[:])

        # ------------ stage 4-6: v (window-major), attention, proj ------------
        # qT/kT/y2T columns are window-major: window w = wi*8+wj occupies
        # cols w*49:(w+1)*49. attT stays spatial-major (scatter on write).

        def win_view(t):
            return t.rearrange("p (a i b j) -> p a b i j", a=8, i=7, b=8, j=7)

        with tc.tile_pool(name="pool_att", bufs=1) as pool_att, \
             tc.tile_pool(name="ps_a", bufs=2, space="PSUM") as ps_a:
            attT = [pool_att.tile([128, N1], BF16, name=f"attT{t}", tag=f"attT{t}")
                    for t in range(4)]
            for wi in range(8):
                vw = pool_vw.tile([49, 8 * C1], BF16, name="vw", tag="vw")
                for wj in range(8):
                    wsl = slice((wi * 8 + wj) * 49, (wi * 8 + wj + 1) * 49)
                    pv = ps_a.tile([49, C1], F32, name="pv", tag="pv")
                    for kt in range(2):
                        nc.tensor.matmul(pv[:], y2T[kt][:, wsl],
                                         wkv[kt][:, C1:2 * C1],
                                         start=(kt == 0), stop=(kt == 1))
                    nc.scalar.copy(out=vw[:, wj * C1:(wj + 1) * C1], in_=pv[:])
                for h in range(8):
                    t, pb_ = h // 2, (h % 2) * 64
                    psl = slice(pb_, pb_ + 64)
                    S = ps_a.tile([49, 392], F32, name="S", tag="S")
                    for wj in range(8):
                        wsl = slice((wi * 8 + wj) * 49, (wi * 8 + wj + 1) * 49)
                        nc.tensor.matmul(S[:, wj * 49:(wj + 1) * 49],
                                         kT[t][psl, wsl],
                                         qT[t][psl, wsl],
                                         start=True, stop=True)
                    E = pool_tmp.tile([49, 392], BF16, name="E", tag="E", bufs=3)
                    nc.scalar.activation(out=E[:], in_=S[:], func=AF.Exp,
                                         scale=0.125)
                    SUMB = ps_a.tile([64, 392], F32, name="SUMB",
                                     tag="SUMB", bufs=1)
                    nc.tensor.matmul(SUMB[:], ones_s[:], E[:],
                                     start=True, stop=True)
                    RB = pool_tmp.tile([64, 392], F32, name="RB", tag="RB", bufs=3)
                    nc.vector.reciprocal(out=RB[:], in_=SUMB[:])
                    AV = ps_a.tile([64, 392], F32, name="AV", tag="AV")
                    for wj in range(8):
                        nc.tensor.matmul(
                            AV[:, wj * 49:(wj + 1) * 49],
                            vw[:, wj * C1 + h * 64:wj * C1 + (h + 1) * 64],
                            E[:, wj * 49:(wj + 1) * 49],
                            start=True, stop=True)
                    avv = AV.rearrange("p (b i j) -> p b i j", b=8, i=7, j=7)
                    rbv = RB.rearrange("p (b i j) -> p b i j", b=8, i=7, j=7)
                    nc.vector.tensor_mul(win_view(attT[t])[psl, wi],
                                         avv[:], rbv[:])

            # ------------ stage 6: output projection ------------
            for nt in range(25):
                nsz = min(128, N1 - nt * 128)
                ns = slice(nt * 128, nt * 128 + nsz)
                po = ps_a.tile([128, C1], F32, name="po", tag="pv")
                for ct in range(4):
                    nc.tensor.matmul(po[:nsz, :], attT[ct][:, ns], wp[ct][:],
                                     start=(ct == 0), stop=False)
                nc.tensor.matmul(po[:nsz, :], ones_r[:, :nsz], bp_sb[:],
                                 start=False, stop=True)
                o_sb = pool_tmp.tile([128, C1], BF16, name="o_sb",
                                     tag="o_sb", bufs=2)
                nc.any.tensor_copy(o_sb[:nsz, :], po[:nsz, :])
                nc.sync.dma_start(out=out[ns, :], in_=o_sb[:nsz, :])


def _get_nc():
    if "nc" not in _cache:
        _cache["nc"] = _build_nc()
    return _cache["nc"]


def kernel(**inputs):
    import ml_dtypes
    bf16 = ml_dtypes.bfloat16
    e4m3 = ml_dtypes.float8_e4m3
    f32 = np.float32

    x = np.asarray(inputs["x"], dtype=f32)
    y = np.asarray(inputs["y"], dtype=f32)
    Wq = np.asarray(inputs["Wq"], dtype=f32)
    Wkv = np.asarray(inputs["Wkv"], dtype=f32)
    Wproj = np.asarray(inputs["Wproj"], dtype=f32)
    bproj = np.asarray(inputs["bproj"], dtype=f32)
    Wsr = np.asarray(inputs["Wsr"], dtype=f32)
    bsr_np = np.asarray(inputs["bsr"], dtype=f32)
    gn = np.asarray(inputs["gn"], dtype=f32)
    bn = np.asarray(inputs["bn"], dtype=f32)

    def win_rows(a):
        # rows: spatial n = (wi*7+i)*56 + wj*7+j  ->  n' = (wi*8+wj)*49 + i*7+j
        c = a.shape[-1]
        v = a.reshape(B, 8, 7, 8, 7, c).transpose(0, 1, 3, 2, 4, 5)
        return np.ascontiguousarray(v.reshape(B, N1, c))

    xw = win_rows(x.astype(e4m3))
    ypool = y.reshape(B, HP, 2, WP, 2, C2).sum(axis=(2, 4))  # sums; Wsr/4 folds mean
    ypw = win_rows(ypool.astype(bf16).reshape(B, HP * WP, C2))

    WqT = np.ascontiguousarray(Wq.T).astype(bf16)
    WsrT = np.ascontiguousarray(0.25 * Wsr.T).astype(bf16)
    WkvT = np.ascontiguousarray(Wkv.T).astype(bf16)
    WpT = np.ascontiguousarray(Wproj.T).astype(bf16)
    gnr = np.ascontiguousarray(gn.reshape(2, 128)).astype(f32)
    bp = np.ascontiguousarray(bproj.reshape(1, C1)).astype(bf16)
    ident = np.eye(128, dtype=bf16)

    nc = _get_nc()
    in_maps = []
    for b in range(B):
        in_maps.append({
            "x8": xw[b], "yp": ypw[b],
            "WqT": WqT, "WsrT": WsrT, "WkvT": WkvT, "WpT": WpT,
            "bsr": bsr_np, "gnr": gnr, "bnc": bn, "bp": bp, "ident": ident,
        })
    from concourse.bass_utils import run_bass_kernel_spmd
    res = run_bass_kernel_spmd(nc, in_maps, core_ids=list(range(B)),
                               **_cache.get("run_opts", {}))
    _cache["last_res"] = res
    return np.stack([r["out"] for r in res.results], axis=0).astype(f32)
